# revision 50
# baseline (speedup 1.0000x reference)
"""Fused GNN-message-passing kernel for Trainium2 (8 NeuronCores, data-parallel).

reference math: for each of B=1024 graphs with 32 nodes, all 496 node pairs
(i<j) in both orderings: features = [onehot_i, onehot_j, PE(pos_j-pos_i),
PE(qinv(q_i) x q_j)] -> 146-dim -> MLP(128,128,128,1) -> 0.003*tanh -> mean
over the two orderings -> out [B, 496].

Structure exploited on device:
- dir-1 features equal dir-0 features up to cos-even / sin-odd symmetry and a
  type-block swap, so trig features are computed ONCE and the two directions
  use different host-precomputed W0 operands.
- pair seeds (pd, lq, onehots) are built token-major with elementwise engines,
  PE-transposed to feature-major, and expanded to angle rows by one sparse
  matmul whose coefficients also fold in the 2^i*pi frequency scaling and the
  1/(2*pi) turns conversion.
- sin/cos via range reduction (magic-number rounding to turns) + ACT Sin.
- biases: b0 rides an all-ones seed row; b1/b2 fused into relu copies; b3 in
  the tanh activation bias.
"""
import numpy as np
import concourse.bass as bass
import concourse.mybir as mybir
import concourse.tile as tile
import concourse.bacc as bacc
from concourse.bass_utils import run_bass_kernel_spmd

F32 = mybir.dt.float32
AF = mybir.ActivationFunctionType
ALU = mybir.AluOpType
MS = bass.MemorySpace
F32R = mybir.dt.float32r
I32 = mybir.dt.int32

B, NN, FDIM, EMB = 1024, 32, 128, 10
NCORES, BC = 8, B // 8          # 128 batches per core
NPAIR = NN * (NN - 1) // 2      # 496
PPB = 8                         # pairs per block / chunk
NBLK = NPAIR // PPB             # 62 chunks
NH = 512                        # MLP half-chunk width (4 pairs x 128 b)
N1 = PPB * BC                   # 1024 tokens per chunk (dir-0 only)
MAGIC = float(np.float32(1.5 * 2 ** 23))
TWO_PI = float(2 * np.pi)
PEN = 0.003


def _host_operands(W0, b0, W3):
    cos_rows = [6 + (r // 3) * 6 + (r % 3) for r in range(30)] + \
               [66 + (rr // 4) * 8 + (rr % 4) for rr in range(40)]
    sin_rows = [6 + (r // 3) * 6 + 3 + (r % 3) for r in range(30)] + \
               [66 + (rr // 4) * 8 + 4 + (rr % 4) for rr in range(40)]
    w0cos0 = W0[cos_rows]                       # [70,128]
    w0sin0 = W0[sin_rows]
    sgn = np.ones(70, np.float32)
    sgn[:30] = -1.0                             # pd flips sign in dir 1
    for rr in range(40):                        # lq vector part flips, w keeps
        sgn[30 + rr] = -1.0 if (rr % 4) != 3 else 1.0
    w0sin1 = w0sin0 * sgn[:, None]
    w0cos = np.concatenate([w0cos0, w0cos0], 1).astype(np.float32)   # [70,256]
    w0sin = np.concatenate([w0sin0, w0sin1], 1).astype(np.float32)
    w0t0 = np.concatenate([W0[0:6], b0[None, :]], 0)                 # [7,128]
    w0t1 = np.concatenate([W0[3:6], W0[0:3], b0[None, :]], 0)
    w0t7 = np.concatenate([w0t0, w0t1], 1).astype(np.float32)        # [7,256]
    w0t = np.zeros((77, 256), np.float32)
    w0t[70:77] = w0t7                    # rows 64..69 stay zero (junk angles)

    asel = np.zeros((128, PPB * 77), np.float32)
    for p in range(PPB):
        for i in range(EMB):
            c = float(2.0 ** (i - 1))           # turns per unit x
            for d in range(3):
                asel[16 * p + d, p * 77 + i * 3 + d] = c
            for q in range(4):
                asel[16 * p + 3 + q, p * 77 + 30 + i * 4 + q] = c
        for c2 in range(6):
            asel[16 * p + 7 + c2, p * 77 + 70 + c2] = 1.0
        asel[16 * p + 13, p * 77 + 76] = 1.0

    return w0cos, w0sin, w0t, asel


def _build_program(b3f, loop=1, variant="full", do_compile=True):
    nc = bacc.Bacc("TRN2", target_bir_lowering=False, debug=False,
                   num_devices=NCORES)
    node_d = nc.dram_tensor("node", [BC, NN * 10], F32, kind="ExternalInput")
    asel_d = nc.dram_tensor("asel", [128, PPB * 77], F32, kind="ExternalInput")
    ident_d = nc.dram_tensor("ident", [128, 128], F32, kind="ExternalInput")
    w0cos_d = nc.dram_tensor("w0cos", [70, 256], F32R, kind="ExternalInput")
    w0t_d = nc.dram_tensor("w0t", [77, 256], F32R, kind="ExternalInput")
    w0sin_d = nc.dram_tensor("w0sin", [70, 256], F32R, kind="ExternalInput")
    w1_d = nc.dram_tensor("w1", [128, 128], F32R, kind="ExternalInput")
    w2_d = nc.dram_tensor("w2", [128, 128], F32R, kind="ExternalInput")
    w3_d = nc.dram_tensor("w3", [128, 1], F32R, kind="ExternalInput")
    b1_d = nc.dram_tensor("b1", [128, 1], F32, kind="ExternalInput")
    b2_d = nc.dram_tensor("b2", [128, 1], F32, kind="ExternalInput")
    b3_d = nc.dram_tensor("b3c", [128, 1], F32, kind="ExternalInput")
    out_d = nc.dram_tensor("out", [BC, NPAIR], F32, kind="ExternalOutput")

    with tile.TileContext(nc) as tc:
        with tc.tile_pool(name="cpool", bufs=1) as cpool, \
             tc.tile_pool(name="wpool", bufs=2) as wpool, \
             tc.tile_pool(name="psA", bufs=1, space=MS.PSUM) as psA, \
             tc.tile_pool(name="psB", bufs=2, space=MS.PSUM) as psB:

            def load_const(name, dram, shape, dtype=F32):
                t = cpool.tile(shape, dtype, tag=name)
                nc.sync.dma_start(t[:], dram[:])
                return t

            node_sb = load_const("node_sb", node_d, [BC, NN * 10])
            asel_sb = load_const("asel_sb", asel_d, [128, PPB * 77])
            ident_sb = load_const("ident_sb", ident_d, [128, 128])
            w0cos_sb = load_const("w0cos_sb", w0cos_d, [70, 256], F32R)
            w0t_sb = load_const("w0t_sb", w0t_d, [77, 256], F32R)
            w0sin_sb = load_const("w0sin_sb", w0sin_d, [70, 256], F32R)
            w1_sb = load_const("w1_sb", w1_d, [128, 128], F32R)
            w2_sb = load_const("w2_sb", w2_d, [128, 128], F32R)
            w3_sb = load_const("w3_sb", w3_d, [128, 1], F32R)
            b1_sb = load_const("b1_sb", b1_d, [128, 1])
            b2_sb = load_const("b2_sb", b2_d, [128, 1])
            b3_sb = load_const("b3_sb", b3_d, [128, 1])

            hpi_sb = cpool.tile([128, 1], F32, tag="hpi_sb")
            nc.vector.memset(hpi_sb[:], float(np.pi / 2))

            w3p_sb = cpool.tile([128, 32 * 128], F32R, tag="w3p_sb")
            nc.vector.tensor_scalar(w3p_sb[:],
                                    w3_sb[:].broadcast_to((128, 32 * 128)),
                                    0.0, None, ALU.mult)
            for vd in range(2):
                for vq in range(16):
                    vcol = (16 * vd + vq) * 128 + 64 * vd + vq
                    nc.vector.tensor_copy(w3p_sb[:, vcol:vcol + 1], w3_sb[:])

            seed = cpool.tile([BC, NPAIR * 16], F32, tag="seed")
            seedT = cpool.tile([128, NBLK * 128], F32, tag="seedT")
            out_sb = cpool.tile([BC, NPAIR], F32, tag="out_sb")

            nodeR = node_sb[:].rearrange("b (k c) -> b k c", c=10)
            seedR = seed[:].rearrange("b (p c) -> b p c", c=16)

            GI = cpool.tile([BC, NPAIR * 10], F32, tag="GI")
            GJ = cpool.tile([BC, NPAIR * 10], F32, tag="GJ")
            T3 = cpool.tile([BC, NPAIR * 3], F32, tag="T3")
            GIr = GI[:].rearrange("b (p c) -> b p c", c=10)
            GJr = GJ[:].rearrange("b (p c) -> b p c", c=10)
            T3r = T3[:].rearrange("b (p c) -> b p c", c=3)

            def _emit_body():
                # cols 13 (ones bias row), 14,15 (transpose reads) need init
                nc.vector.memset(seedR[:, :, 13:16], 1.0)

                def stage_b_gather(ilo, ihi, pbase):
                    for i in range(ilo, ihi):
                        J = NN - 1 - i
                        sl = slice(pbase, pbase + J)
                        nc.gpsimd.tensor_copy(GJr[:, sl, :],
                                              nodeR[:, i + 1:, :])
                        nc.vector.tensor_copy(
                            GIr[:, sl, :],
                            nodeR[:, i:i + 1, :].broadcast_to((BC, J, 10)))
                        pbase += J
                    return pbase

                def stage_b_compute(plo, phi):
                    ps = slice(plo, phi)
                    NP = phi - plo
                    QIr, QJr = GIr, GJr
                    QIv, QJv = QIr[:, ps, 0:3], QJr[:, ps, 0:3]
                    wIb = QIr[:, ps, 3:4].broadcast_to((BC, NP, 3))
                    wJb = QJr[:, ps, 3:4].broadcast_to((BC, NP, 3))
                    nc.vector.tensor_copy(seedR[:, ps, 7:10], GIr[:, ps, 7:10])
                    nc.vector.tensor_copy(seedR[:, ps, 10:13], GJr[:, ps, 7:10])
                    SV = seedR[:, ps, 3:6]
                    SW = seedR[:, ps, 6:7]
                    nc.vector.tensor_sub(seedR[:, ps, 0:3], GJr[:, ps, 4:7],
                                         GIr[:, ps, 4:7])
                    nc.vector.tensor_tensor(SV, QJv, wIb, ALU.mult)
                    nc.vector.tensor_tensor(T3r[:, ps, :], QIv, wJb, ALU.mult)
                    nc.vector.tensor_sub(SV, SV, T3r[:, ps, :])
                    for c in range(3):
                        c1, c2 = (c + 1) % 3, (c + 2) % 3
                        svc = SV[:, :, c:c + 1]
                        t1c = T3r[:, ps, 0:1]
                        nc.vector.tensor_tensor(
                            t1c, QJr[:, ps, c1:c1 + 1], QIr[:, ps, c2:c2 + 1],
                            ALU.mult)
                        nc.vector.tensor_tensor(svc, svc, t1c, ALU.add)
                        nc.vector.tensor_tensor(
                            t1c, QJr[:, ps, c2:c2 + 1], QIr[:, ps, c1:c1 + 1],
                            ALU.mult)
                        nc.vector.tensor_tensor(svc, svc, t1c, ALU.subtract)
                    nc.vector.tensor_tensor(SW, QIr[:, ps, 3:4],
                                            QJr[:, ps, 3:4], ALU.mult)
                    nc.vector.tensor_tensor(T3r[:, ps, :], QIv, QJv, ALU.mult)
                    for c in range(3):
                        nc.vector.tensor_tensor(SW, SW, T3r[:, ps, c:c + 1],
                                                ALU.add)

                PSPLIT = 243    # i blocks 0..8
                if "noseed" not in variant:
                    pb = stage_b_gather(0, 9, 0)
                    stage_b_compute(0, PSPLIT)

                # ---- per-chunk, software-pipelined: the front-end of
                #      chunk t+1 is emitted before the MLP of chunk t so the
                #      per-engine FIFOs interleave the two stages ----
                RS = 8                        # chunks per z-round
                state = {"zpack": None}

                def emit_front(t):
                    tp = psB.tile([128, 128], F32, tag="u_psh")
                    nc.tensor.transpose(tp[:], seed[:, t * 128:(t + 1) * 128],
                                        ident_sb[:])
                    nc.vector.tensor_copy(seedT[:, t * 128:(t + 1) * 128], tp[:])
                    xcos = wpool.tile([70, N1], F32R, tag="xcos", bufs=3)
                    xsin = wpool.tile([70, N1], F32R, tag="xsin")
                    f = wpool.tile([70, N1], F32, tag="f")
                    f2c = wpool.tile([70, N1], F32, tag="f2c")
                    rs = []
                    for hh in range(2):
                        cs = slice(hh * NH, (hh + 1) * NH)
                        u_ps = psB.tile([77, NH], F32, tag="u_psh")
                        for pl in range(4):
                            p = hh * 4 + pl
                            nc.tensor.matmul(
                                u_ps[:, pl * BC:(pl + 1) * BC],
                                asel_sb[:, p * 77:(p + 1) * 77],
                                seedT[:, t * 128:(t + 1) * 128],
                                start=True, stop=True)
                        # round(u): ang rows get the frac base; rows 70..76
                        # hold one-hots/ones which round() reproduces exactly,
                        # so r doubles as the W0t matmul operand.
                        r = wpool.tile([77, NH], F32R, tag="r", bufs=4)
                        nc.vector.tensor_scalar(r[:], u_ps[:], MAGIC, MAGIC,
                                                ALU.add, ALU.subtract)
                        rf = r[0:70, :].bitcast(F32)
                        nc.vector.tensor_sub(f[:, cs], u_ps[0:70, :], rf)
                        rs.append(r)
                    nc.vector.tensor_scalar(
                        f2c[:].bitcast(I32), f[:].bitcast(I32),
                        0x7FFFFFFF, None, ALU.bitwise_and)
                    nc.scalar.activation(xsin[:], f[:], AF.Sin, scale=TWO_PI)
                    nc.scalar.activation(xcos[:], f2c[:], AF.Sin,
                                         scale=-TWO_PI, bias=hpi_sb[0:70])
                    return xcos, xsin, rs

                def emit_mlp(t, xcos, xsin, rs):
                    rnd, tl = t // RS, t % RS
                    TL = min(RS, NBLK - RS * rnd)
                    if tl == 0:
                        zpack_t = psA.tile([128, NH], F32, tag="zpack")
                        state["zpack"] = zpack_t
                    zpack = state["zpack"]
                    for h in range(2):
                        cs = slice(h * NH, (h + 1) * NH)
                        h1p = psA.tile([128, 2 * NH], F32, tag="h1p")
                        h1 = wpool.tile([128, 2 * NH], F32R, tag="h1")
                        for d in range(2):
                            ds = slice(d * 128, (d + 1) * 128)
                            dsl = slice(d * NH, (d + 1) * NH)
                            h0p = psB.tile([128, NH], F32, tag="h0p")
                            nc.tensor.matmul(h0p[:], w0cos_sb[:, ds], xcos[:, cs],
                                             start=True, stop=False)
                            nc.tensor.matmul(h0p[:], w0sin_sb[:, ds], xsin[:, cs],
                                             start=False, stop=False)
                            nc.tensor.matmul(h0p[:], w0t_sb[64:77, ds],
                                             rs[h][64:77, :],
                                             start=False, stop=True)
                            h0 = wpool.tile([128, NH], F32R, tag="h0", bufs=4)
                            if d == 0:
                                nc.vector.tensor_scalar_max(h0[:], h0p[:], 0.0)
                            else:
                                nc.scalar.activation(h0[:], h0p[:], AF.Relu)
                            nc.tensor.matmul(h1p[:, dsl], w1_sb[:], h0[:],
                                             start=True, stop=True)
                        nc.scalar.activation(h1[:], h1p[:], AF.Relu,
                                             bias=b1_sb[:])
                        for d in range(2):
                            dsl = slice(d * NH, (d + 1) * NH)
                            h2p = psA.tile([128, NH], F32, tag="h2p")
                            nc.tensor.matmul(h2p[:], w2_sb[:], h1[:, dsl],
                                             start=True, stop=True)
                            h2 = wpool.tile([128, NH], F32R, tag="h2", bufs=4)
                            if d == 0:
                                nc.vector.tensor_scalar(h2[:], h2p[:], b2_sb[:],
                                                        0.0, ALU.add, ALU.max)
                            else:
                                nc.scalar.activation(h2[:], h2p[:], AF.Relu,
                                                     bias=b2_sb[:])
                            v = 16 * d + 2 * tl + h
                            nc.tensor.matmul(
                                zpack[:],
                                w3p_sb[:, v * 128:(v + 1) * 128],
                                h2[:],
                                start=(tl == 0 and h == 0 and d == 0),
                                stop=(h == 1 and d == 1 and tl == TL - 1))

                    # round flush: tanh, mean over dirs, scatter to out
                    if tl == TL - 1:
                        CL = TL
                        zS = wpool.tile([128, NH], F32, tag="zS")
                        nc.vector.tensor_copy(zS[:], zpack[:])
                        outv = out_sb[:].rearrange("b (q g) -> b q g", g=4)
                        for g in range(4):
                            ztP = psB.tile([128, 128], F32, tag="u_psh")
                            nc.tensor.transpose(ztP[:],
                                                zS[:, g * 128:(g + 1) * 128],
                                                ident_sb[:])
                            ztS = wpool.tile([128, 128], F32, tag="ztS")
                            nc.scalar.activation(ztS[:], ztP[:], AF.Tanh,
                                                 bias=b3_sb[:])
                            ztmp = wpool.tile([128, 32], F32, tag="ztmp")
                            nc.vector.tensor_tensor(
                                ztmp[:, 0:2 * CL], ztS[:, 0:2 * CL],
                                ztS[:, 64:64 + 2 * CL], ALU.add)
                            vv = outv[:, RS * 2 * rnd:RS * 2 * rnd + 2 * CL,
                                      g:g + 1]
                            tmpv = ztmp[:].rearrange(
                                "b (q one) -> b q one", one=1)[:, 0:2 * CL, :]
                            nc.vector.tensor_scalar(vv, tmpv, PEN * 0.5, None,
                                                    ALU.mult)

                fr = emit_front(0)
                for t in range(NBLK):
                    fr_next = emit_front(t + 1) if t + 1 < NBLK else None
                    if "nomlp" not in variant:
                        emit_mlp(t, *fr)
                    if t == 0 and "noseed" not in variant:
                        stage_b_gather(9, NN - 1, PSPLIT)
                        stage_b_compute(PSPLIT, NPAIR)
                    fr = fr_next

                if "nomlp" in variant or "noz" in variant or "noh12" in variant:
                    nc.vector.memset(out_sb[:], 0.0)
                nc.sync.dma_start(out_d[:], out_sb[:])

            if loop == 1:
                _emit_body()
            else:
                with tc.For_i(0, loop, 1):
                    _emit_body()
    if do_compile:
        nc.compile()
    return nc


_PROGRAM_CACHE = {}


def _get_program(b3f, loop=1, variant="full", do_compile=True):
    key = (b3f, loop, variant)
    if key not in _PROGRAM_CACHE:
        _PROGRAM_CACHE[key] = _build_program(b3f, loop, variant, do_compile)
    return _PROGRAM_CACHE[key]


def make_in_maps(obj_type, gparam, pos, quat, W0, b0, W1, b1, W2, b2, W3, b3,
                 **_unused):
    del gparam
    W0 = np.asarray(W0, np.float32)
    b0 = np.asarray(b0, np.float32)
    W3 = np.asarray(W3, np.float32).reshape(128, 1)
    w0cos, w0sin, w0t, asel = _host_operands(W0, b0, W3)
    shared = {
        "asel": asel,
        "ident": np.eye(128, dtype=np.float32),
        "w0cos": w0cos,
        "w0sin": w0sin,
        "w0t": w0t,
        "w1": np.ascontiguousarray(W1, np.float32),
        "w2": np.ascontiguousarray(W2, np.float32),
        "w3": np.ascontiguousarray(W3, np.float32),
        "b1": np.asarray(b1, np.float32).reshape(128, 1),
        "b2": np.asarray(b2, np.float32).reshape(128, 1),
        "b3c": np.full((128, 1), float(np.float32(b3).reshape(-1)[0]),
                       np.float32),
    }
    obj_type = np.asarray(obj_type, np.float32)
    pos = np.asarray(pos, np.float32)
    quat = np.asarray(quat, np.float32)
    in_maps = []
    for c in range(NCORES):
        s = slice(c * BC, (c + 1) * BC)
        node = np.concatenate(
            [quat[s], pos[s], obj_type[s]], axis=2).reshape(BC, NN * 10)
        in_maps.append({"node": np.ascontiguousarray(node), **shared})
    return in_maps


def kernel(**inputs):
    in_maps = make_in_maps(**inputs)
    nc = _get_program(float(np.float32(inputs["b3"]).reshape(-1)[0]))
    res = run_bass_kernel_spmd(nc, in_maps, core_ids=list(range(NCORES)))
    out = np.concatenate([res.results[c]["out"] for c in range(NCORES)], axis=0)
    return np.ascontiguousarray(out, np.float32)



# revision 51
# speedup vs baseline: 1.0703x; 1.0703x over previous
"""Fused GNN-message-passing kernel for Trainium2 (8 NeuronCores, data-parallel).

reference math: for each of B=1024 graphs with 32 nodes, all 496 node pairs
(i<j) in both orderings: features = [onehot_i, onehot_j, PE(pos_j-pos_i),
PE(qinv(q_i) x q_j)] -> 146-dim -> MLP(128,128,128,1) -> 0.003*tanh -> mean
over the two orderings -> out [B, 496].

Structure exploited on device:
- dir-1 features equal dir-0 features up to cos-even / sin-odd symmetry and a
  type-block swap, so trig features are computed ONCE and the two directions
  use different host-precomputed W0 operands.
- pair seeds (pd, lq, onehots) are built token-major with elementwise engines,
  PE-transposed to feature-major, and expanded to angle rows by one sparse
  matmul whose coefficients also fold in the 2^i*pi frequency scaling and the
  1/(2*pi) turns conversion.
- sin/cos via range reduction (magic-number rounding to turns) + ACT Sin.
- biases: b0 rides an all-ones seed row; b1/b2 fused into relu copies; b3 in
  the tanh activation bias.
"""
import numpy as np
import concourse.bass as bass
import concourse.mybir as mybir
import concourse.tile as tile
import concourse.bacc as bacc
from concourse.bass_utils import run_bass_kernel_spmd

F32 = mybir.dt.float32
AF = mybir.ActivationFunctionType
ALU = mybir.AluOpType
MS = bass.MemorySpace
F32R = mybir.dt.float32r
I32 = mybir.dt.int32

B, NN, FDIM, EMB = 1024, 32, 128, 10
NCORES, BC = 8, B // 8          # 128 batches per core
NPAIR = NN * (NN - 1) // 2      # 496
PPB = 8                         # pairs per block / chunk
NBLK = NPAIR // PPB             # 62 chunks
NH = 512                        # MLP half-chunk width (4 pairs x 128 b)
N1 = PPB * BC                   # 1024 tokens per chunk (dir-0 only)
MAGIC = float(np.float32(1.5 * 2 ** 23))
TWO_PI = float(2 * np.pi)
PEN = 0.003


def _host_operands(W0, b0, W3):
    cos_rows = [6 + (r // 3) * 6 + (r % 3) for r in range(30)] + \
               [66 + (rr // 4) * 8 + (rr % 4) for rr in range(40)]
    sin_rows = [6 + (r // 3) * 6 + 3 + (r % 3) for r in range(30)] + \
               [66 + (rr // 4) * 8 + 4 + (rr % 4) for rr in range(40)]
    w0cos0 = W0[cos_rows]                       # [70,128]
    w0sin0 = W0[sin_rows]
    sgn = np.ones(70, np.float32)
    sgn[:30] = -1.0                             # pd flips sign in dir 1
    for rr in range(40):                        # lq vector part flips, w keeps
        sgn[30 + rr] = -1.0 if (rr % 4) != 3 else 1.0
    w0sin1 = w0sin0 * sgn[:, None]
    w0cos = np.concatenate([w0cos0, w0cos0], 1).astype(np.float32)   # [70,256]
    w0sin = np.concatenate([w0sin0, w0sin1], 1).astype(np.float32)
    w0t0 = np.concatenate([W0[0:6], b0[None, :]], 0)                 # [7,128]
    w0t1 = np.concatenate([W0[3:6], W0[0:3], b0[None, :]], 0)
    w0t7 = np.concatenate([w0t0, w0t1], 1).astype(np.float32)        # [7,256]
    w0cos = np.concatenate([w0cos, w0t7], 0)   # [77,256]

    asel = np.zeros((128, PPB * 77), np.float32)
    for p in range(PPB):
        for i in range(EMB):
            c = float(2.0 ** (i - 1))           # turns per unit x
            for d in range(3):
                asel[16 * p + d, p * 77 + i * 3 + d] = c
            for q in range(4):
                asel[16 * p + 3 + q, p * 77 + 30 + i * 4 + q] = c
        for c2 in range(6):
            asel[16 * p + 7 + c2, p * 77 + 70 + c2] = 1.0
        asel[16 * p + 13, p * 77 + 76] = 1.0

    return w0cos, w0sin, asel


def _build_program(b3f, loop=1, variant="full", do_compile=True):
    nc = bacc.Bacc("TRN2", target_bir_lowering=False, debug=False,
                   num_devices=NCORES)
    node_d = nc.dram_tensor("node", [BC, NN * 10], F32, kind="ExternalInput")
    asel_d = nc.dram_tensor("asel", [128, PPB * 77], F32, kind="ExternalInput")
    ident_d = nc.dram_tensor("ident", [128, 128], F32, kind="ExternalInput")
    w0cos_d = nc.dram_tensor("w0cos", [77, 256], F32R, kind="ExternalInput")
    w0sin_d = nc.dram_tensor("w0sin", [70, 256], F32R, kind="ExternalInput")
    w1_d = nc.dram_tensor("w1", [128, 128], F32R, kind="ExternalInput")
    w2_d = nc.dram_tensor("w2", [128, 128], F32R, kind="ExternalInput")
    w3_d = nc.dram_tensor("w3", [128, 1], F32R, kind="ExternalInput")
    b1_d = nc.dram_tensor("b1", [128, 1], F32, kind="ExternalInput")
    b2_d = nc.dram_tensor("b2", [128, 1], F32, kind="ExternalInput")
    b3_d = nc.dram_tensor("b3c", [128, 1], F32, kind="ExternalInput")
    out_d = nc.dram_tensor("out", [BC, NPAIR], F32, kind="ExternalOutput")

    with tile.TileContext(nc) as tc:
        with tc.tile_pool(name="cpool", bufs=1) as cpool, \
             tc.tile_pool(name="wpool", bufs=2) as wpool, \
             tc.tile_pool(name="psA", bufs=1, space=MS.PSUM) as psA, \
             tc.tile_pool(name="psB", bufs=2, space=MS.PSUM) as psB:

            def load_const(name, dram, shape, dtype=F32):
                t = cpool.tile(shape, dtype, tag=name)
                nc.sync.dma_start(t[:], dram[:])
                return t

            node_sb = load_const("node_sb", node_d, [BC, NN * 10])
            asel_sb = load_const("asel_sb", asel_d, [128, PPB * 77])
            ident_sb = load_const("ident_sb", ident_d, [128, 128])
            w0cos_sb = load_const("w0cos_sb", w0cos_d, [77, 256], F32R)
            w0sin_sb = load_const("w0sin_sb", w0sin_d, [70, 256], F32R)
            w1_sb = load_const("w1_sb", w1_d, [128, 128], F32R)
            w2_sb = load_const("w2_sb", w2_d, [128, 128], F32R)
            w3_sb = load_const("w3_sb", w3_d, [128, 1], F32R)
            b1_sb = load_const("b1_sb", b1_d, [128, 1])
            b2_sb = load_const("b2_sb", b2_d, [128, 1])
            b3_sb = load_const("b3_sb", b3_d, [128, 1])

            hpi_sb = cpool.tile([128, 1], F32, tag="hpi_sb")
            nc.vector.memset(hpi_sb[:], float(np.pi / 2))

            w3p_sb = cpool.tile([128, 32 * 128], F32R, tag="w3p_sb")
            nc.vector.tensor_scalar(w3p_sb[:],
                                    w3_sb[:].broadcast_to((128, 32 * 128)),
                                    0.0, None, ALU.mult)
            for vd in range(2):
                for vq in range(16):
                    vcol = (16 * vd + vq) * 128 + 64 * vd + vq
                    nc.vector.tensor_copy(w3p_sb[:, vcol:vcol + 1], w3_sb[:])

            seed = cpool.tile([BC, NPAIR * 16], F32, tag="seed")
            seedT = cpool.tile([128, NBLK * 128], F32, tag="seedT")
            out_sb = cpool.tile([BC, NPAIR], F32, tag="out_sb")

            nodeR = node_sb[:].rearrange("b (k c) -> b k c", c=10)
            seedR = seed[:].rearrange("b (p c) -> b p c", c=16)

            GI = cpool.tile([BC, NPAIR * 10], F32, tag="GI")
            GJ = cpool.tile([BC, NPAIR * 10], F32, tag="GJ")
            T3 = cpool.tile([BC, NPAIR * 3], F32, tag="T3")
            GIr = GI[:].rearrange("b (p c) -> b p c", c=10)
            GJr = GJ[:].rearrange("b (p c) -> b p c", c=10)
            T3r = T3[:].rearrange("b (p c) -> b p c", c=3)

            def _emit_body():
                # cols 13 (ones bias row), 14,15 (transpose reads) need init
                nc.vector.memset(seedR[:, :, 13:16], 1.0)

                def stage_b_gather(ilo, ihi, pbase):
                    for i in range(ilo, ihi):
                        J = NN - 1 - i
                        sl = slice(pbase, pbase + J)
                        nc.gpsimd.tensor_copy(GJr[:, sl, :],
                                              nodeR[:, i + 1:, :])
                        nc.vector.tensor_copy(
                            GIr[:, sl, :],
                            nodeR[:, i:i + 1, :].broadcast_to((BC, J, 10)))
                        pbase += J
                    return pbase

                def stage_b_compute(plo, phi):
                    ps = slice(plo, phi)
                    NP = phi - plo
                    QIr, QJr = GIr, GJr
                    QIv, QJv = QIr[:, ps, 0:3], QJr[:, ps, 0:3]
                    wIb = QIr[:, ps, 3:4].broadcast_to((BC, NP, 3))
                    wJb = QJr[:, ps, 3:4].broadcast_to((BC, NP, 3))
                    nc.vector.tensor_copy(seedR[:, ps, 7:10], GIr[:, ps, 7:10])
                    nc.vector.tensor_copy(seedR[:, ps, 10:13], GJr[:, ps, 7:10])
                    SV = seedR[:, ps, 3:6]
                    SW = seedR[:, ps, 6:7]
                    nc.vector.tensor_sub(seedR[:, ps, 0:3], GJr[:, ps, 4:7],
                                         GIr[:, ps, 4:7])
                    nc.vector.tensor_tensor(SV, QJv, wIb, ALU.mult)
                    nc.vector.tensor_tensor(T3r[:, ps, :], QIv, wJb, ALU.mult)
                    nc.vector.tensor_sub(SV, SV, T3r[:, ps, :])
                    for c in range(3):
                        c1, c2 = (c + 1) % 3, (c + 2) % 3
                        svc = SV[:, :, c:c + 1]
                        t1c = T3r[:, ps, 0:1]
                        nc.vector.tensor_tensor(
                            t1c, QJr[:, ps, c1:c1 + 1], QIr[:, ps, c2:c2 + 1],
                            ALU.mult)
                        nc.vector.tensor_tensor(svc, svc, t1c, ALU.add)
                        nc.vector.tensor_tensor(
                            t1c, QJr[:, ps, c2:c2 + 1], QIr[:, ps, c1:c1 + 1],
                            ALU.mult)
                        nc.vector.tensor_tensor(svc, svc, t1c, ALU.subtract)
                    nc.vector.tensor_tensor(SW, QIr[:, ps, 3:4],
                                            QJr[:, ps, 3:4], ALU.mult)
                    nc.vector.tensor_tensor(T3r[:, ps, :], QIv, QJv, ALU.mult)
                    for c in range(3):
                        nc.vector.tensor_tensor(SW, SW, T3r[:, ps, c:c + 1],
                                                ALU.add)

                PSPLIT = 243    # i blocks 0..8
                if "noseed" not in variant:
                    pb = stage_b_gather(0, 9, 0)
                    stage_b_compute(0, PSPLIT)

                # ---- per-chunk, software-pipelined: the front-end of
                #      chunk t+1 is emitted before the MLP of chunk t so the
                #      per-engine FIFOs interleave the two stages ----
                RS = 8                        # chunks per z-round
                state = {"zpack": None}

                def emit_front(t):
                    tp = psB.tile([128, 128], F32, tag="u_psh")
                    nc.tensor.transpose(tp[:], seed[:, t * 128:(t + 1) * 128],
                                        ident_sb[:])
                    nc.vector.tensor_copy(seedT[:, t * 128:(t + 1) * 128], tp[:])
                    xcos = wpool.tile([77, N1], F32R, tag="xcos", bufs=3)
                    xsin = wpool.tile([70, N1], F32R, tag="xsin")
                    f = wpool.tile([70, N1], F32, tag="f")
                    f2c = wpool.tile([70, N1], F32, tag="f2c")
                    rs = []
                    for hh in range(2):
                        cs = slice(hh * NH, (hh + 1) * NH)
                        u_ps = psB.tile([77, NH], F32, tag="u_psh")
                        for pl in range(4):
                            p = hh * 4 + pl
                            nc.tensor.matmul(
                                u_ps[:, pl * BC:(pl + 1) * BC],
                                asel_sb[:, p * 77:(p + 1) * 77],
                                seedT[:, t * 128:(t + 1) * 128],
                                start=True, stop=True)
                        # round(u): ang rows get the frac base; rows 70..76
                        # hold one-hots/ones which round() reproduces exactly,
                        # so r doubles as the W0t matmul operand.
                        r = wpool.tile([70, NH], F32R, tag="r", bufs=4)
                        nc.vector.tensor_scalar(r[:], u_ps[0:70, :], MAGIC,
                                                MAGIC, ALU.add, ALU.subtract)
                        rf = r[:].bitcast(F32)
                        nc.vector.tensor_sub(f[:, cs], u_ps[0:70, :], rf)
                        if hh == 0:
                            nc.vector.tensor_copy(xcos[64:77, cs],
                                                  u_ps[64:77, :])
                        else:
                            nc.scalar.activation(xcos[64:77, cs],
                                                 u_ps[64:77, :], AF.Copy)
                        rs.append(r)
                    nc.vector.tensor_scalar(
                        f2c[:].bitcast(I32), f[:].bitcast(I32),
                        0x7FFFFFFF, None, ALU.bitwise_and)
                    nc.scalar.activation(xsin[:], f[:], AF.Sin, scale=TWO_PI)
                    nc.scalar.activation(xcos[0:70, :], f2c[:], AF.Sin,
                                         scale=-TWO_PI, bias=hpi_sb[0:70])
                    return xcos, xsin, rs

                def emit_mlp(t, xcos, xsin, rs):
                    rnd, tl = t // RS, t % RS
                    TL = min(RS, NBLK - RS * rnd)
                    if tl == 0:
                        zpack_t = psA.tile([128, NH], F32, tag="zpack")
                        state["zpack"] = zpack_t
                    zpack = state["zpack"]
                    for h in range(2):
                        cs = slice(h * NH, (h + 1) * NH)
                        h1p = psA.tile([128, 2 * NH], F32, tag="h1p")
                        h1 = wpool.tile([128, 2 * NH], F32R, tag="h1")
                        for d in range(2):
                            ds = slice(d * 128, (d + 1) * 128)
                            dsl = slice(d * NH, (d + 1) * NH)
                            h0p = psB.tile([128, NH], F32, tag="h0p")
                            nc.tensor.matmul(h0p[:], w0cos_sb[:, ds], xcos[:, cs],
                                             start=True, stop=False)
                            nc.tensor.matmul(h0p[:], w0sin_sb[:, ds], xsin[:, cs],
                                             start=False, stop=True)
                            h0 = wpool.tile([128, NH], F32R, tag="h0", bufs=4)
                            if d == 0:
                                nc.vector.tensor_scalar_max(h0[:], h0p[:], 0.0)
                            else:
                                nc.scalar.activation(h0[:], h0p[:], AF.Relu)
                            nc.tensor.matmul(h1p[:, dsl], w1_sb[:], h0[:],
                                             start=True, stop=True)
                        nc.scalar.activation(h1[:], h1p[:], AF.Relu,
                                             bias=b1_sb[:])
                        for d in range(2):
                            dsl = slice(d * NH, (d + 1) * NH)
                            h2p = psA.tile([128, NH], F32, tag="h2p")
                            nc.tensor.matmul(h2p[:], w2_sb[:], h1[:, dsl],
                                             start=True, stop=True)
                            h2 = wpool.tile([128, NH], F32R, tag="h2", bufs=4)
                            if d == 0:
                                nc.vector.tensor_scalar(h2[:], h2p[:], b2_sb[:],
                                                        0.0, ALU.add, ALU.max)
                            else:
                                nc.scalar.activation(h2[:], h2p[:], AF.Relu,
                                                     bias=b2_sb[:])
                            v = 16 * d + 2 * tl + h
                            nc.tensor.matmul(
                                zpack[:],
                                w3p_sb[:, v * 128:(v + 1) * 128],
                                h2[:],
                                start=(tl == 0 and h == 0 and d == 0),
                                stop=(h == 1 and d == 1 and tl == TL - 1))

                    # round flush: tanh, mean over dirs, scatter to out
                    if tl == TL - 1:
                        CL = TL
                        zS = wpool.tile([128, NH], F32, tag="zS")
                        nc.vector.tensor_copy(zS[:], zpack[:])
                        outv = out_sb[:].rearrange("b (q g) -> b q g", g=4)
                        for g in range(4):
                            ztP = psB.tile([128, 128], F32, tag="u_psh")
                            nc.tensor.transpose(ztP[:],
                                                zS[:, g * 128:(g + 1) * 128],
                                                ident_sb[:])
                            ztS = wpool.tile([128, 128], F32, tag="ztS")
                            nc.scalar.activation(ztS[:], ztP[:], AF.Tanh,
                                                 bias=b3_sb[:])
                            ztmp = wpool.tile([128, 32], F32, tag="ztmp")
                            nc.vector.tensor_tensor(
                                ztmp[:, 0:2 * CL], ztS[:, 0:2 * CL],
                                ztS[:, 64:64 + 2 * CL], ALU.add)
                            vv = outv[:, RS * 2 * rnd:RS * 2 * rnd + 2 * CL,
                                      g:g + 1]
                            tmpv = ztmp[:].rearrange(
                                "b (q one) -> b q one", one=1)[:, 0:2 * CL, :]
                            nc.vector.tensor_scalar(vv, tmpv, PEN * 0.5, None,
                                                    ALU.mult)

                fr = emit_front(0)
                for t in range(NBLK):
                    fr_next = emit_front(t + 1) if t + 1 < NBLK else None
                    if "nomlp" not in variant:
                        emit_mlp(t, *fr)
                    if t == 0 and "noseed" not in variant:
                        stage_b_gather(9, NN - 1, PSPLIT)
                        stage_b_compute(PSPLIT, NPAIR)
                    fr = fr_next

                if "nomlp" in variant or "noz" in variant or "noh12" in variant:
                    nc.vector.memset(out_sb[:], 0.0)
                nc.sync.dma_start(out_d[:], out_sb[:])

            if loop == 1:
                _emit_body()
            else:
                with tc.For_i(0, loop, 1):
                    _emit_body()
    if do_compile:
        nc.compile()
    return nc


_PROGRAM_CACHE = {}


def _get_program(b3f, loop=1, variant="full", do_compile=True):
    key = (b3f, loop, variant)
    if key not in _PROGRAM_CACHE:
        _PROGRAM_CACHE[key] = _build_program(b3f, loop, variant, do_compile)
    return _PROGRAM_CACHE[key]


def make_in_maps(obj_type, gparam, pos, quat, W0, b0, W1, b1, W2, b2, W3, b3,
                 **_unused):
    del gparam
    W0 = np.asarray(W0, np.float32)
    b0 = np.asarray(b0, np.float32)
    W3 = np.asarray(W3, np.float32).reshape(128, 1)
    w0cos, w0sin, asel = _host_operands(W0, b0, W3)
    shared = {
        "asel": asel,
        "ident": np.eye(128, dtype=np.float32),
        "w0cos": w0cos,
        "w0sin": w0sin,
        "w1": np.ascontiguousarray(W1, np.float32),
        "w2": np.ascontiguousarray(W2, np.float32),
        "w3": np.ascontiguousarray(W3, np.float32),
        "b1": np.asarray(b1, np.float32).reshape(128, 1),
        "b2": np.asarray(b2, np.float32).reshape(128, 1),
        "b3c": np.full((128, 1), float(np.float32(b3).reshape(-1)[0]),
                       np.float32),
    }
    obj_type = np.asarray(obj_type, np.float32)
    pos = np.asarray(pos, np.float32)
    quat = np.asarray(quat, np.float32)
    in_maps = []
    for c in range(NCORES):
        s = slice(c * BC, (c + 1) * BC)
        node = np.concatenate(
            [quat[s], pos[s], obj_type[s]], axis=2).reshape(BC, NN * 10)
        in_maps.append({"node": np.ascontiguousarray(node), **shared})
    return in_maps


def kernel(**inputs):
    in_maps = make_in_maps(**inputs)
    nc = _get_program(float(np.float32(inputs["b3"]).reshape(-1)[0]))
    res = run_bass_kernel_spmd(nc, in_maps, core_ids=list(range(NCORES)))
    out = np.concatenate([res.results[c]["out"] for c in range(NCORES)], axis=0)
    return np.ascontiguousarray(out, np.float32)



# revision 53
# speedup vs baseline: 1.0817x; 1.0107x over previous
"""Fused GNN-message-passing kernel for Trainium2 (8 NeuronCores, data-parallel).

reference math: for each of B=1024 graphs with 32 nodes, all 496 node pairs
(i<j) in both orderings: features = [onehot_i, onehot_j, PE(pos_j-pos_i),
PE(qinv(q_i) x q_j)] -> 146-dim -> MLP(128,128,128,1) -> 0.003*tanh -> mean
over the two orderings -> out [B, 496].

Structure exploited on device:
- dir-1 features equal dir-0 features up to cos-even / sin-odd symmetry and a
  type-block swap, so trig features are computed ONCE and the two directions
  use different host-precomputed W0 operands.
- pair seeds (pd, lq, onehots) are built token-major with elementwise engines,
  PE-transposed to feature-major, and expanded to angle rows by one sparse
  matmul whose coefficients also fold in the 2^i*pi frequency scaling and the
  1/(2*pi) turns conversion.
- sin/cos via range reduction (magic-number rounding to turns) + ACT Sin.
- biases: b0 rides an all-ones seed row; b1/b2 fused into relu copies; b3 in
  the tanh activation bias.
"""
import numpy as np
import concourse.bass as bass
import concourse.mybir as mybir
import concourse.tile as tile
import concourse.bacc as bacc
from concourse.bass_utils import run_bass_kernel_spmd

F32 = mybir.dt.float32
AF = mybir.ActivationFunctionType
ALU = mybir.AluOpType
MS = bass.MemorySpace
F32R = mybir.dt.float32r
I32 = mybir.dt.int32

B, NN, FDIM, EMB = 1024, 32, 128, 10
NCORES, BC = 8, B // 8          # 128 batches per core
NPAIR = NN * (NN - 1) // 2      # 496
PPB = 8                         # pairs per block / chunk
NBLK = NPAIR // PPB             # 62 chunks
NH = 512                        # MLP half-chunk width (4 pairs x 128 b)
N1 = PPB * BC                   # 1024 tokens per chunk (dir-0 only)
MAGIC = float(np.float32(1.5 * 2 ** 23))
TWO_PI = float(2 * np.pi)
PEN = 0.003


def _host_operands(W0, b0, W3):
    cos_rows = [6 + (r // 3) * 6 + (r % 3) for r in range(30)] + \
               [66 + (rr // 4) * 8 + (rr % 4) for rr in range(40)]
    sin_rows = [6 + (r // 3) * 6 + 3 + (r % 3) for r in range(30)] + \
               [66 + (rr // 4) * 8 + 4 + (rr % 4) for rr in range(40)]
    w0cos0 = W0[cos_rows]                       # [70,128]
    w0sin0 = W0[sin_rows]
    sgn = np.ones(70, np.float32)
    sgn[:30] = -1.0                             # pd flips sign in dir 1
    for rr in range(40):                        # lq vector part flips, w keeps
        sgn[30 + rr] = -1.0 if (rr % 4) != 3 else 1.0
    w0sin1 = w0sin0 * sgn[:, None]
    w0cos = np.concatenate([w0cos0, w0cos0], 1).astype(np.float32)   # [70,256]
    w0sin = np.concatenate([w0sin0, w0sin1], 1).astype(np.float32)
    w0t0 = np.concatenate([W0[0:6], b0[None, :]], 0)                 # [7,128]
    w0t1 = np.concatenate([W0[3:6], W0[0:3], b0[None, :]], 0)
    w0t7 = np.concatenate([w0t0, w0t1], 1).astype(np.float32)        # [7,256]
    w0cos = np.concatenate([w0cos, w0t7], 0)   # [77,256]

    asel = np.zeros((128, PPB * 77), np.float32)
    for p in range(PPB):
        for i in range(EMB):
            c = float(2.0 ** (i - 1))           # turns per unit x
            for d in range(3):
                asel[16 * p + d, p * 77 + i * 3 + d] = c
            for q in range(4):
                asel[16 * p + 3 + q, p * 77 + 30 + i * 4 + q] = c
        for c2 in range(6):
            asel[16 * p + 7 + c2, p * 77 + 70 + c2] = 1.0
        asel[16 * p + 13, p * 77 + 76] = 1.0

    return w0cos, w0sin, asel


def _build_program(b3f, loop=1, variant="full", do_compile=True):
    nc = bacc.Bacc("TRN2", target_bir_lowering=False, debug=False,
                   num_devices=NCORES)
    node_d = nc.dram_tensor("node", [BC, NN * 10], F32, kind="ExternalInput")
    asel_d = nc.dram_tensor("asel", [128, PPB * 77], F32, kind="ExternalInput")
    ident_d = nc.dram_tensor("ident", [128, 128], F32, kind="ExternalInput")
    w0cos_d = nc.dram_tensor("w0cos", [77, 256], F32R, kind="ExternalInput")
    w0sin_d = nc.dram_tensor("w0sin", [70, 256], F32R, kind="ExternalInput")
    w1_d = nc.dram_tensor("w1", [128, 128], F32R, kind="ExternalInput")
    w2_d = nc.dram_tensor("w2", [128, 128], F32R, kind="ExternalInput")
    w3_d = nc.dram_tensor("w3", [128, 1], F32R, kind="ExternalInput")
    b1_d = nc.dram_tensor("b1", [128, 1], F32, kind="ExternalInput")
    b2_d = nc.dram_tensor("b2", [128, 1], F32, kind="ExternalInput")
    b3_d = nc.dram_tensor("b3c", [128, 1], F32, kind="ExternalInput")
    out_d = nc.dram_tensor("out", [BC, NPAIR], F32, kind="ExternalOutput")

    with tile.TileContext(nc) as tc:
        with tc.tile_pool(name="cpool", bufs=1) as cpool, \
             tc.tile_pool(name="wpool", bufs=2) as wpool, \
             tc.tile_pool(name="psA", bufs=1, space=MS.PSUM) as psA, \
             tc.tile_pool(name="psB", bufs=2, space=MS.PSUM) as psB:

            def load_const(name, dram, shape, dtype=F32):
                t = cpool.tile(shape, dtype, tag=name)
                nc.sync.dma_start(t[:], dram[:])
                return t

            node_sb = load_const("node_sb", node_d, [BC, NN * 10])
            asel_sb = load_const("asel_sb", asel_d, [128, PPB * 77])
            ident_sb = load_const("ident_sb", ident_d, [128, 128])
            w0cos_sb = load_const("w0cos_sb", w0cos_d, [77, 256], F32R)
            w0sin_sb = load_const("w0sin_sb", w0sin_d, [70, 256], F32R)
            w1_sb = load_const("w1_sb", w1_d, [128, 128], F32R)
            w2_sb = load_const("w2_sb", w2_d, [128, 128], F32R)
            w3_sb = load_const("w3_sb", w3_d, [128, 1], F32R)
            b1_sb = load_const("b1_sb", b1_d, [128, 1])
            b2_sb = load_const("b2_sb", b2_d, [128, 1])
            b3_sb = load_const("b3_sb", b3_d, [128, 1])

            hpi_sb = cpool.tile([128, 1], F32, tag="hpi_sb")
            nc.vector.memset(hpi_sb[:], float(np.pi / 2))

            w3p_sb = cpool.tile([128, 32 * 128], F32R, tag="w3p_sb")
            nc.vector.tensor_scalar(w3p_sb[:],
                                    w3_sb[:].broadcast_to((128, 32 * 128)),
                                    0.0, None, ALU.mult)
            for vd in range(2):
                for vq in range(16):
                    vcol = (16 * vd + vq) * 128 + 64 * vd + vq
                    nc.vector.tensor_copy(w3p_sb[:, vcol:vcol + 1], w3_sb[:])

            seed = cpool.tile([BC, NPAIR * 16], F32, tag="seed")
            seedT = cpool.tile([128, NBLK * 128], F32, tag="seedT")
            out_sb = cpool.tile([BC, NPAIR], F32, tag="out_sb")

            nodeR = node_sb[:].rearrange("b (k c) -> b k c", c=10)
            seedR = seed[:].rearrange("b (p c) -> b p c", c=16)

            GI = cpool.tile([BC, NPAIR * 10], F32, tag="GI")
            GJ = cpool.tile([BC, NPAIR * 10], F32, tag="GJ")
            T3 = cpool.tile([BC, NPAIR * 3], F32, tag="T3")
            GIr = GI[:].rearrange("b (p c) -> b p c", c=10)
            GJr = GJ[:].rearrange("b (p c) -> b p c", c=10)
            T3r = T3[:].rearrange("b (p c) -> b p c", c=3)

            def _emit_body():
                # cols 13 (ones bias row), 14,15 (transpose reads) need init
                nc.vector.memset(seedR[:, :, 13:16], 1.0)

                def stage_b_gather(ilo, ihi, pbase):
                    for i in range(ilo, ihi):
                        J = NN - 1 - i
                        sl = slice(pbase, pbase + J)
                        nc.gpsimd.tensor_copy(GJr[:, sl, :],
                                              nodeR[:, i + 1:, :])
                        nc.vector.tensor_copy(
                            GIr[:, sl, :],
                            nodeR[:, i:i + 1, :].broadcast_to((BC, J, 10)))
                        pbase += J
                    return pbase

                def stage_b_compute(plo, phi):
                    ps = slice(plo, phi)
                    NP = phi - plo
                    QIr, QJr = GIr, GJr
                    QIv, QJv = QIr[:, ps, 0:3], QJr[:, ps, 0:3]
                    wIb = QIr[:, ps, 3:4].broadcast_to((BC, NP, 3))
                    wJb = QJr[:, ps, 3:4].broadcast_to((BC, NP, 3))
                    nc.vector.tensor_copy(seedR[:, ps, 7:10], GIr[:, ps, 7:10])
                    nc.vector.tensor_copy(seedR[:, ps, 10:13], GJr[:, ps, 7:10])
                    SV = seedR[:, ps, 3:6]
                    SW = seedR[:, ps, 6:7]
                    nc.vector.tensor_sub(seedR[:, ps, 0:3], GJr[:, ps, 4:7],
                                         GIr[:, ps, 4:7])
                    nc.vector.tensor_tensor(SV, QJv, wIb, ALU.mult)
                    nc.vector.tensor_tensor(T3r[:, ps, :], QIv, wJb, ALU.mult)
                    nc.vector.tensor_sub(SV, SV, T3r[:, ps, :])
                    for c in range(3):
                        c1, c2 = (c + 1) % 3, (c + 2) % 3
                        svc = SV[:, :, c:c + 1]
                        t1c = T3r[:, ps, 0:1]
                        nc.vector.tensor_tensor(
                            t1c, QJr[:, ps, c1:c1 + 1], QIr[:, ps, c2:c2 + 1],
                            ALU.mult)
                        nc.vector.tensor_tensor(svc, svc, t1c, ALU.add)
                        nc.vector.tensor_tensor(
                            t1c, QJr[:, ps, c2:c2 + 1], QIr[:, ps, c1:c1 + 1],
                            ALU.mult)
                        nc.vector.tensor_tensor(svc, svc, t1c, ALU.subtract)
                    nc.vector.tensor_tensor(SW, QIr[:, ps, 3:4],
                                            QJr[:, ps, 3:4], ALU.mult)
                    nc.vector.tensor_tensor(T3r[:, ps, :], QIv, QJv, ALU.mult)
                    for c in range(3):
                        nc.vector.tensor_tensor(SW, SW, T3r[:, ps, c:c + 1],
                                                ALU.add)

                PSPLIT = 243    # i blocks 0..8
                if "noseed" not in variant:
                    pb = stage_b_gather(0, 9, 0)
                    stage_b_compute(0, PSPLIT)

                # ---- per-chunk, software-pipelined: the front-end of
                #      chunk t+1 is emitted before the MLP of chunk t so the
                #      per-engine FIFOs interleave the two stages ----
                RS = 8                        # chunks per z-round
                state = {"zpack": None}

                def emit_front(t):
                    tp = psB.tile([128, 128], F32, tag="u_psh")
                    nc.tensor.transpose(tp[:], seed[:, t * 128:(t + 1) * 128],
                                        ident_sb[:])
                    nc.vector.tensor_copy(seedT[:, t * 128:(t + 1) * 128], tp[:])
                    xcos = wpool.tile([77, N1], F32R, tag="xcos", bufs=3)
                    xsin = wpool.tile([70, N1], F32R, tag="xsin")
                    f = wpool.tile([70, N1], F32, tag="f")
                    f2c = wpool.tile([70, N1], F32, tag="f2c")
                    rs = []
                    for hh in range(2):
                        cs = slice(hh * NH, (hh + 1) * NH)
                        u_ps = psB.tile([77, NH], F32, tag="u_psh")
                        for pl in range(4):
                            p = hh * 4 + pl
                            nc.tensor.matmul(
                                u_ps[:, pl * BC:(pl + 1) * BC],
                                asel_sb[:, p * 77:(p + 1) * 77],
                                seedT[:, t * 128:(t + 1) * 128],
                                start=True, stop=True)
                        # round(u): ang rows get the frac base; rows 70..76
                        # hold one-hots/ones which round() reproduces exactly,
                        # so r doubles as the W0t matmul operand.
                        r = wpool.tile([70, NH], F32R, tag="r", bufs=4)
                        nc.vector.tensor_scalar(r[:], u_ps[0:70, :], MAGIC,
                                                MAGIC, ALU.add, ALU.subtract)
                        rf = r[:].bitcast(F32)
                        nc.vector.tensor_sub(f[:, cs], u_ps[0:70, :], rf)
                        if hh == 0:
                            nc.vector.tensor_copy(xcos[64:77, cs],
                                                  u_ps[64:77, :])
                        else:
                            nc.scalar.activation(xcos[64:77, cs],
                                                 u_ps[64:77, :], AF.Copy)
                        rs.append(r)
                    nc.vector.tensor_scalar(
                        f2c[:].bitcast(I32), f[:].bitcast(I32),
                        0x7FFFFFFF, None, ALU.bitwise_and)
                    nc.scalar.activation(xsin[:], f[:], AF.Sin, scale=TWO_PI)
                    nc.scalar.activation(xcos[0:70, :], f2c[:], AF.Sin,
                                         scale=-TWO_PI, bias=hpi_sb[0:70])
                    return xcos, xsin, rs

                def emit_mlp(t, xcos, xsin, rs):
                    rnd, tl = t // RS, t % RS
                    TL = min(RS, NBLK - RS * rnd)
                    if tl == 0:
                        zpack_t = psA.tile([128, NH], F32, tag="zpack")
                        state["zpack"] = zpack_t
                    zpack = state["zpack"]
                    for h in range(2):
                        cs = slice(h * NH, (h + 1) * NH)
                        h1p = psA.tile([128, 2 * NH], F32, tag="h1p")
                        h1 = wpool.tile([128, 2 * NH], F32R, tag="h1")
                        for d in range(2):
                            ds = slice(d * 128, (d + 1) * 128)
                            dsl = slice(d * NH, (d + 1) * NH)
                            h0p = psB.tile([128, NH], F32, tag="h0p")
                            nc.tensor.matmul(h0p[:], w0cos_sb[:, ds], xcos[:, cs],
                                             start=True, stop=False)
                            nc.tensor.matmul(h0p[:], w0sin_sb[:, ds], xsin[:, cs],
                                             start=False, stop=True)
                            h0 = wpool.tile([128, NH], F32R, tag="h0", bufs=4)
                            if d == 0:
                                nc.vector.tensor_scalar_max(h0[:], h0p[:], 0.0)
                            else:
                                nc.scalar.activation(h0[:], h0p[:], AF.Relu)
                            nc.tensor.matmul(h1p[:, dsl], w1_sb[:], h0[:],
                                             start=True, stop=True)
                        nc.scalar.activation(h1[:], h1p[:], AF.Relu,
                                             bias=b1_sb[:])
                        for d in range(2):
                            dsl = slice(d * NH, (d + 1) * NH)
                            h2p = psA.tile([128, NH], F32, tag="h2p")
                            nc.tensor.matmul(h2p[:], w2_sb[:], h1[:, dsl],
                                             start=True, stop=True)
                            h2 = wpool.tile([128, NH], F32R, tag="h2", bufs=4)
                            if d == 0:
                                nc.vector.tensor_scalar(h2[:], h2p[:], b2_sb[:],
                                                        0.0, ALU.add, ALU.max)
                            else:
                                nc.scalar.activation(h2[:], h2p[:], AF.Relu,
                                                     bias=b2_sb[:])
                            v = 16 * d + 2 * tl + h
                            nc.tensor.matmul(
                                zpack[:],
                                w3p_sb[:, v * 128:(v + 1) * 128],
                                h2[:],
                                start=(tl == 0 and h == 0 and d == 0),
                                stop=(h == 1 and d == 1 and tl == TL - 1))

                    # round flush: tanh, mean over dirs, scatter to out
                    if tl == TL - 1:
                        CL = TL
                        zS = wpool.tile([128, NH], F32, tag="zS")
                        nc.vector.tensor_copy(zS[:], zpack[:])
                        outv = out_sb[:].rearrange("b (q g) -> b q g", g=4)
                        for g in range(4):
                            ztP = psB.tile([128, 128], F32, tag="u_psh")
                            nc.tensor.transpose(ztP[:],
                                                zS[:, g * 128:(g + 1) * 128],
                                                ident_sb[:])
                            ztS = wpool.tile([128, 128], F32, tag="ztS")
                            nc.scalar.activation(ztS[:], ztP[:], AF.Tanh,
                                                 bias=b3_sb[:])
                            ztmp = wpool.tile([128, 32], F32, tag="ztmp")
                            nc.vector.tensor_tensor(
                                ztmp[:, 0:2 * CL], ztS[:, 0:2 * CL],
                                ztS[:, 64:64 + 2 * CL], ALU.add)
                            vv = outv[:, RS * 2 * rnd:RS * 2 * rnd + 2 * CL,
                                      g:g + 1]
                            tmpv = ztmp[:].rearrange(
                                "b (q one) -> b q one", one=1)[:, 0:2 * CL, :]
                            nc.vector.tensor_scalar(vv, tmpv, PEN * 0.5, None,
                                                    ALU.mult)

                fr = emit_front(0)
                for t in range(NBLK):
                    fr_next = emit_front(t + 1) if t + 1 < NBLK else None
                    if "nomlp" not in variant:
                        emit_mlp(t, *fr)
                    if t == 0 and "noseed" not in variant:
                        stage_b_gather(9, NN - 1, PSPLIT)
                        stage_b_compute(PSPLIT, NPAIR)
                    fr = fr_next

                if "nomlp" in variant or "noz" in variant or "noh12" in variant:
                    nc.vector.memset(out_sb[:], 0.0)
                nc.sync.dma_start(out_d[:], out_sb[:])

            if loop == 1:
                _emit_body()
            else:
                with tc.For_i(0, loop, 1):
                    _emit_body()
    if do_compile:
        nc.compile()
    return nc


_PROGRAM_CACHE = {}


def _get_program(b3f, loop=1, variant="full", do_compile=True):
    key = (b3f, loop, variant)
    if key not in _PROGRAM_CACHE:
        _PROGRAM_CACHE[key] = _build_program(b3f, loop, variant, do_compile)
    return _PROGRAM_CACHE[key]


def make_in_maps(obj_type, gparam, pos, quat, W0, b0, W1, b1, W2, b2, W3, b3,
                 **_unused):
    del gparam
    W0 = np.asarray(W0, np.float32)
    b0 = np.asarray(b0, np.float32)
    W3 = np.asarray(W3, np.float32).reshape(128, 1)
    w0cos, w0sin, asel = _host_operands(W0, b0, W3)
    shared = {
        "asel": asel,
        "ident": np.eye(128, dtype=np.float32),
        "w0cos": w0cos,
        "w0sin": w0sin,
        "w1": np.ascontiguousarray(W1, np.float32),
        "w2": np.ascontiguousarray(W2, np.float32),
        "w3": np.ascontiguousarray(W3, np.float32),
        "b1": np.asarray(b1, np.float32).reshape(128, 1),
        "b2": np.asarray(b2, np.float32).reshape(128, 1),
        "b3c": np.full((128, 1), float(np.float32(b3).reshape(-1)[0]),
                       np.float32),
    }
    obj_type = np.asarray(obj_type, np.float32)
    pos = np.asarray(pos, np.float32)
    quat = np.asarray(quat, np.float32)
    in_maps = []
    for c in range(NCORES):
        s = slice(c * BC, (c + 1) * BC)
        node = np.concatenate(
            [quat[s], pos[s], obj_type[s]], axis=2).reshape(BC, NN * 10)
        in_maps.append({"node": np.ascontiguousarray(node), **shared})
    return in_maps


def kernel(**inputs):
    in_maps = make_in_maps(**inputs)
    nc = _get_program(float(np.float32(inputs["b3"]).reshape(-1)[0]))
    res = run_bass_kernel_spmd(nc, in_maps, core_ids=list(range(NCORES)))
    out = np.concatenate([res.results[c]["out"] for c in range(NCORES)], axis=0)
    return np.ascontiguousarray(out, np.float32)

